# revision 1
# baseline (speedup 1.0000x reference)
"""Bi-directional correlation cost volume on 8 Trainium2 NeuronCores.

Strategy (data-parallel over batch, one batch element per core):
  - Per core, compute the Gram band G[u, x] = sum_c L[c,h,u] * R[c,h,x] / C
    for |x - u| <= 63 with TensorE matmuls (K=C=32, 4x row-tiled over
    h-groups so 4 matmuls share the PE array).
  - Stage the band rectangles to HBM as [h, chunk, u, x-window].
  - The cost volume out[d, x] = G[x -/+ d, x] is a *shear* of the band;
    host extracts the 127 diagonals with one vectorized gather per batch.
"""

import numpy as np

B, C, H, WIMG, D = 8, 32, 160, 320, 64
# (u0, U, xw0, W): u-chunk start/size, x-window start/size
CHUNKS = [(0, 128, 0, 191), (128, 128, 65, 254), (256, 64, 193, 127)]
WSLOT = 256
# staging: per h, chunk ci is a [128, WSLOT] slot; row u holds W valid elems
WPAD = [WSLOT, WSLOT, WSLOT]
COFF = [0, 128 * WSLOT, 2 * 128 * WSLOT]
HROW = 3 * 128 * WSLOT
HQ = H // 4  # h-rows per PE quadrant

_CACHE = {}


HGRP = 16      # h-rows batched per store DMA
ACT_MOD = 3    # every ACT_MOD-th copy goes to ScalarE (0 = all DVE)
STAG_BF16 = False  # stage the Gram band in bf16 (halves store traffic)


def _get_nc(reps=1):
    key = ("nc", reps, HGRP, ACT_MOD, STAG_BF16)
    if key in _CACHE:
        return _CACHE[key]
    import concourse.bacc as bacc
    import concourse.tile as tile
    from concourse import mybir

    f32 = mybir.dt.float32
    sdt = mybir.dt.bfloat16 if STAG_BF16 else f32
    nc = bacc.Bacc("TRN2", target_bir_lowering=False, debug=False)
    r_in = nc.declare_dram_parameter("r_in", [C, H, WIMG], f32, isOutput=False)
    l_in = nc.declare_dram_parameter("l_in", [C, H, WIMG], f32, isOutput=False)
    stag = nc.declare_dram_parameter("stag", [H, HROW], sdt, isOutput=True)

    with tile.TileContext(nc) as tc:
        with tc.tile_pool(name="inp", bufs=1) as inp_pool, \
             tc.tile_pool(name="ps", bufs=6, space="PSUM") as ps_pool, \
             tc.tile_pool(name="st", bufs=6) as st_pool:
            Lsb = inp_pool.tile([128, HQ * WIMG], f32, tag="L")
            Rsb = inp_pool.tile([128, HQ * WIMG], f32, tag="R")
            # partition (q, c) holds h-rows [40q, 40q+40) of channel c
            for q in range(4):
                nc.sync.dma_start(
                    Lsb[32 * q:32 * (q + 1), :],
                    l_in[:, HQ * q:HQ * (q + 1), :].rearrange(
                        "c hh x -> c (hh x)"),
                )
                nc.sync.dma_start(
                    Rsb[32 * q:32 * (q + 1), :],
                    r_in[:, HQ * q:HQ * (q + 1), :].rearrange(
                        "c hh x -> c (hh x)"),
                )
            for _ in range(reps):
                for q in range(4):
                    for hh0 in range(0, HQ, HGRP):
                        G = min(HGRP, HQ - hh0)
                        for ci, (u0, U, xw0, W) in enumerate(CHUNKS):
                            sb = st_pool.tile([128, HGRP * WSLOT], sdt,
                                              tag="sb")
                            for g in range(G):
                                hh = hh0 + g
                                ps = ps_pool.tile([128, 256], f32, tag="ps")
                                nc.tensor.matmul(
                                    ps[:U, :W],
                                    Lsb[32 * q:32 * (q + 1),
                                        hh * WIMG + u0:hh * WIMG + u0 + U],
                                    Rsb[32 * q:32 * (q + 1),
                                        hh * WIMG + xw0:hh * WIMG + xw0 + W],
                                    start=True, stop=True,
                                    tile_position=(32 * q, 0),
                                )
                                dst = sb[:U, g * WSLOT:g * WSLOT + W]
                                if ACT_MOD and hh % ACT_MOD == ACT_MOD - 1:
                                    nc.scalar.mul(dst, ps[:U, :W], 1.0 / C)
                                else:
                                    nc.vector.tensor_scalar_mul(
                                        dst, ps[:U, :W], 1.0 / C)
                            h0 = HQ * q + hh0
                            dma_eng = nc.sync if ci % 2 else nc.scalar
                            dst_ap = stag[h0:h0 + G,
                                          COFF[ci]:COFF[ci] + U * WPAD[ci]]
                            dma_eng.dma_start(
                                dst_ap.rearrange(
                                    "g (u w) -> u g w", u=U)[:, :, :W],
                                sb[:U, :].rearrange(
                                    "u (g w) -> u g w", g=HGRP)[:, :G, :W],
                            )
    nc.compile()
    _CACHE[key] = nc
    return nc


def _gather_idx():
    if "idx" in _CACHE:
        return _CACHE["idx"]
    P_ = np.arange(2 * D)[:, None]
    dts = np.where(P_ < D, P_, -(P_ - D))  # signed disparity per output plane
    x = np.arange(WIMG)[None, :]
    u = np.clip(x - dts, 0, WIMG - 1)
    c = np.minimum(u // 128, 2)
    u0 = c * 128
    xw0 = np.choose(c, [ch[2] for ch in CHUNKS])
    Wc = np.choose(c, [ch[3] for ch in CHUNKS])
    wp = np.choose(c, WPAD)
    off = np.choose(c, COFF)
    w = np.clip(x - xw0, 0, Wc - 1)
    idx2d = off + (u - u0) * wp + w
    _CACHE["idx"] = np.ascontiguousarray(idx2d.reshape(-1).astype(np.int64))
    return _CACHE["idx"]


def _assemble(stag_b):
    """stag_b: [H, HROW] packed band -> out_b [2D, H, WIMG]"""
    idx = _gather_idx()
    flat = np.asarray(stag_b).astype(np.float32).reshape(H, -1)
    o = np.empty((H, 2 * D, WIMG), dtype=np.float32)
    ov = o.reshape(H, -1)
    for h in range(H):
        np.take(flat[h], idx, out=ov[h])
    o = np.ascontiguousarray(o.transpose(1, 0, 2))
    for d in range(1, D):
        o[d, :, :d] = 0
        o[D + d, :, WIMG - d:] = 0
    return o


def run_cores(right_np, left_np, timing_reps=0):
    """Run the SPMD bass kernel; returns (list of staging arrays, exec_ns)."""
    from concourse.bass_utils import run_bass_kernel_spmd

    nc = _get_nc()
    in_maps = [
        {"r_in": np.ascontiguousarray(right_np[b]),
         "l_in": np.ascontiguousarray(left_np[b])}
        for b in range(B)
    ]
    res = run_bass_kernel_spmd(nc, in_maps, list(range(B)))
    return [res.results[b]["stag"] for b in range(B)]


def kernel(right_feature, left_feature, max_disp):
    assert int(max_disp) == D
    right_np = np.asarray(right_feature, dtype=np.float32)
    left_np = np.asarray(left_feature, dtype=np.float32)
    stags = run_cores(right_np, left_np)
    out = np.stack([_assemble(s) for s in stags])
    return out



# revision 4
# speedup vs baseline: 1.9775x; 1.9775x over previous
"""Bi-directional correlation cost volume on 8 Trainium2 NeuronCores.

Strategy (data-parallel over batch, one batch element per core):
  - Per core, compute the Gram band G[u, x] = sum_c L[c,h,u] * R[c,h,x] / C
    for |x - u| <= 63 with TensorE matmuls in bf16 (K=C=32, 4x row-tiled
    over h-quadrants via tile_position; quadrant-interleaved issue order so
    the four 32-row PE sub-arrays run concurrently).
  - Matmuls pack 2-4 h-rows per PSUM bank; one DVE/ACT copy per bank
    (scale 1/C, cast to bf16) into SBUF staging tiles.
  - Stage the band to HBM as three chunk regions [U, H, W] (bf16, tight),
    so each store DMA writes per-partition-contiguous runs of G*W*2 bytes.
  - The cost volume out[p, x] = G[x - k(p), x] is a shear of the band; the
    host extracts it with one precomputed vectorized gather per batch.
"""

import numpy as np

B, C, H, WIMG, D = 8, 32, 160, 320, 64
# (u0, U, xw0, W): u-chunk start/size, x-window start/size
CHUNKS = [(0, 128, 0, 191), (128, 128, 65, 254), (256, 64, 193, 127)]
SIZES = [U * H * W for (_, U, _, W) in CHUNKS]
OFFS = [0, SIZES[0], SIZES[0] + SIZES[1]]
NTOT = sum(SIZES)
HQ = H // 4   # h-rows per PE quadrant
HGRP = 16     # h-rows per staging tile / store DMA
WMAX = max(W for (_, _, _, W) in CHUNKS)

_CACHE = {}


def _bf16():
    from concourse import mybir
    return mybir.dt.np(mybir.dt.bfloat16)


def _get_nc(reps=1):
    key = ("nc", reps, HGRP)
    if key in _CACHE:
        return _CACHE[key]
    import concourse.bacc as bacc
    import concourse.tile as tile
    from concourse import mybir

    f32 = mybir.dt.float32
    bf16 = mybir.dt.bfloat16
    nc = bacc.Bacc("TRN2", target_bir_lowering=False, debug=False)
    r_in = nc.declare_dram_parameter("r_in", [C, H, WIMG], bf16, isOutput=False)
    l_in = nc.declare_dram_parameter("l_in", [C, H, WIMG], bf16, isOutput=False)
    stags = [
        nc.declare_dram_parameter(f"stag{ci}", [U, H, W], bf16, isOutput=True)
        for ci, (_, U, _, W) in enumerate(CHUNKS)
    ]

    with tile.TileContext(nc) as tc:
        with tc.tile_pool(name="inp", bufs=1) as inp_pool, \
             tc.tile_pool(name="ps", bufs=8, space="PSUM") as ps_pool, \
             tc.tile_pool(name="st", bufs=6) as st_pool:
            Lsb = inp_pool.tile([128, HQ * WIMG], bf16, tag="L")
            Rsb = inp_pool.tile([128, HQ * WIMG], bf16, tag="R")
            # partition (q, c) holds h-rows [40q, 40q+40) of channel c
            for q in range(4):
                nc.sync.dma_start(
                    Lsb[32 * q:32 * (q + 1), :],
                    l_in[:, HQ * q:HQ * (q + 1), :].rearrange(
                        "c hh x -> c (hh x)"),
                )
                nc.sync.dma_start(
                    Rsb[32 * q:32 * (q + 1), :],
                    r_in[:, HQ * q:HQ * (q + 1), :].rearrange(
                        "c hh x -> c (hh x)"),
                )
            cctr = 0  # copy-engine round robin
            dctr = 0  # dma-engine round robin
            for _ in range(reps):
                for hh0 in range(0, HQ, HGRP):
                    G = min(HGRP, HQ - hh0)
                    for ci, (u0, U, xw0, W) in enumerate(CHUNKS):
                        # h-rows packed per PSUM bank (512 f32/partition)
                        P = 512 // W
                        sbs = [st_pool.tile([128, HGRP * WMAX], bf16,
                                            tag="sb", name=f"sb{_q}")
                               for _q in range(4)]
                        for pi in range(G // P):
                            pss = [ps_pool.tile([128, 512], f32, tag="ps",
                                                name=f"ps{_q}")
                                   for _q in range(4)]
                            # quadrant-interleaved so the 4 row-tiles of the
                            # PE array have concurrent work
                            for s in range(P):
                                for q in range(4):
                                    hh = hh0 + pi * P + s
                                    base = hh * WIMG
                                    nc.tensor.matmul(
                                        pss[q][:U, s * W:(s + 1) * W],
                                        Lsb[32 * q:32 * (q + 1),
                                            base + u0:base + u0 + U],
                                        Rsb[32 * q:32 * (q + 1),
                                            base + xw0:base + xw0 + W],
                                        start=True, stop=True,
                                        tile_position=(32 * q, 0),
                                    )
                            for q in range(4):
                                dst = sbs[q][:U, pi * P * W:(pi * P + P) * W]
                                src = pss[q][:U, :P * W]
                                if cctr % 2 == 0:
                                    nc.scalar.mul(dst, src, 1.0 / C)
                                else:
                                    nc.vector.tensor_scalar_mul(
                                        dst, src, 1.0 / C)
                                cctr += 1
                        for q in range(4):
                            h0 = HQ * q + hh0
                            dst = stags[ci][:, h0:h0 + G, :]
                            src = sbs[q][:U, :G * W].rearrange(
                                "u (g w) -> u g w", g=G)
                            eng = nc.sync if dctr % 2 else nc.scalar
                            dctr += 1
                            eng.dma_start(dst, src)
    nc.compile()
    _CACHE[key] = nc
    return nc


def _gather_idx():
    """IDX [2D, H, W] into the concatenated staging flat; valid mask."""
    if "idx" in _CACHE:
        return _CACHE["idx"]
    p = np.arange(2 * D)[:, None]
    k = np.where(p < D, p, -(p - D))      # signed disparity per plane
    x = np.arange(WIMG)[None, :]
    u = x - k                             # [2D, W]
    valid = (u >= 0) & (u < WIMG)
    uc = np.clip(u, 0, WIMG - 1)
    c = np.minimum(uc // 128, 2)
    u0 = np.choose(c, [ch[0] for ch in CHUNKS])
    xw0 = np.choose(c, [ch[2] for ch in CHUNKS])
    Wc = np.choose(c, [ch[3] for ch in CHUNKS])
    off = np.choose(c, OFFS)
    w = np.clip(x - xw0, 0, Wc - 1)
    base = off + (uc - u0) * (H * Wc) + w          # [2D, W], h=0
    idx = (base[:, None, :]
           + np.arange(H)[None, :, None] * Wc[:, None, :]).astype(np.int32)
    vmask = valid[:, None, :].astype(np.float32)   # [2D, 1, W]
    _CACHE["idx"] = (idx, vmask)
    return _CACHE["idx"]


def _assemble(stag_b):
    """stag_b: tuple of 3 chunk arrays (bf16) -> out_b [2D, H, W] f32."""
    idx, vmask = _gather_idx()
    flat = np.concatenate([np.asarray(s).ravel() for s in stag_b])
    out = flat[idx].astype(np.float32)
    out *= vmask
    return out


def run_cores(right_np, left_np, timing_reps=0):
    """Run the SPMD bass kernel; returns list of per-batch staging tuples."""
    from concourse.bass_utils import run_bass_kernel_spmd

    bf = _bf16()
    nc = _get_nc()
    in_maps = [
        {"r_in": np.ascontiguousarray(right_np[b]).astype(bf),
         "l_in": np.ascontiguousarray(left_np[b]).astype(bf)}
        for b in range(B)
    ]
    res = run_bass_kernel_spmd(nc, in_maps, list(range(B)))
    return [tuple(res.results[b][f"stag{ci}"] for ci in range(3))
            for b in range(B)]


def kernel(right_feature, left_feature, max_disp):
    assert int(max_disp) == D
    right_np = np.asarray(right_feature, dtype=np.float32)
    left_np = np.asarray(left_feature, dtype=np.float32)
    stags = run_cores(right_np, left_np)
    out = np.stack([_assemble(s) for s in stags])
    return out
